# revision 16
# baseline (speedup 1.0000x reference)
"""Multi-head self-attention with LoRA on 8 Trainium2 NeuronCores.

Sharding: core c -> (batch b = c//2, head-half hh = c%2).
LoRA is folded into the weights on the host (W' = W + 0.5*A@B, exact).
Each core:
  - projects its 8 heads' q/k (transposed layout) and v (natural layout)
    for all 2048 tokens of its batch, in bf16 (fp32 PSUM accumulation)
  - attention for its 8 heads over all 2048 queries; softmax denominators
    via DVE pairwise tree + one ones-matmul (partition sum)
  - an 8-core AllGather shares all cores' attention outputs, then the
    O-projection is resharded by output dim: each core computes its 256
    output dims for all 8192 (batch, token) pairs, written transposed
Host: weight folding/transposes/bf16 casts and output assembly.
"""

import os
import numpy as np
import ml_dtypes

import concourse.bacc as bacc
import concourse.mybir as mybir
import concourse.tile as tile
from concourse.bass_utils import run_bass_kernel_spmd

F32 = mybir.dt.float32
F32R = mybir.dt.float32r
BF16 = mybir.dt.bfloat16
AF = mybir.ActivationFunctionType
BF = ml_dtypes.bfloat16

B, L, D = 4, 2048, 2048
H, HD = 16, 128
SCALING = 0.5          # lora alpha / rank
SCALE = HD ** -0.5     # attention score scale
P = 128                # partitions
NT = D // P            # 16 tiles along the full feature dim
HL = D // 2            # 1024: head-half feature dim per core
NH = 8                 # heads per core
TT = L // P            # 16 tiles along token dim
CH = 512               # moving-dim chunk (queries / tokens)
VC = 256               # v-projection dout chunk
NCORES = 8
OD = D // NCORES       # 256: O-projection output dims per core
ODT = OD // P          # 2 output-dim tiles per core

_cache = {}


def _build():
    nc = bacc.Bacc(num_devices=NCORES)

    xt = nc.dram_tensor("xt", [D, L], BF16, kind="ExternalInput")
    wq = nc.dram_tensor("wq", [D, HL], BF16, kind="ExternalInput")
    wk = nc.dram_tensor("wk", [D, HL], BF16, kind="ExternalInput")
    wv = nc.dram_tensor("wv", [D, HL], BF16, kind="ExternalInput")
    wo = nc.dram_tensor("wo", [D, OD], BF16, kind="ExternalInput")
    bq = nc.dram_tensor("bq", [HL], F32, kind="ExternalInput")
    bk = nc.dram_tensor("bk", [HL], F32, kind="ExternalInput")
    bv = nc.dram_tensor("bv", [1, HL], BF16, kind="ExternalInput")
    bo = nc.dram_tensor("bo", [OD], F32, kind="ExternalInput")
    yt = nc.dram_tensor("yt", [OD, B * L], F32, kind="ExternalOutput")

    ones_f_d = nc.inline_tensor(np.ones((P, P), dtype=np.float32), name="ones_f_d")
    ones_b_d = nc.inline_tensor(np.ones((1, P), dtype=BF), name="ones_b_d")
    ones_c_d = nc.inline_tensor(np.ones((P, 1), dtype=BF), name="ones_c_d")

    def dma(out, in_):
        nc.sync.dma_start(out=out, in_=in_)

    def r(ap):
        return ap.bitcast(F32R)

    with tile.TileContext(nc) as tc:
        with (
            tc.tile_pool(name="consts", bufs=1) as consts,
            tc.tile_pool(name="qk_sb", bufs=1) as qkpool,
            tc.tile_pool(name="v_sb", bufs=1) as vpool,
            tc.tile_pool(name="dram", bufs=1, space="DRAM") as dpool,
        ):
            # ---- persistent constants ----
            ones_f = consts.tile([P, P], F32, tag="ones_f")
            nc.sync.dma_start(out=ones_f.bitcast(F32R),
                              in_=ones_f_d[:, :].bitcast(F32R))
            ones_b = consts.tile([1, P], BF16, tag="ones_b")
            dma(ones_b, ones_b_d[:, :])
            ones_c = consts.tile([P, 1], BF16, tag="ones_c")
            dma(ones_c, ones_c_d[:, :])
            # q/k biases as per-partition scalars [128, head_tile]
            biasq = consts.tile([P, NH], F32, tag="biasq")
            dma(biasq, bq[:].rearrange("(t p) -> p t", p=P))
            biask = consts.tile([P, NH], F32, tag="biask")
            dma(biask, bk[:].rearrange("(t p) -> p t", p=P))
            biaso = consts.tile([P, ODT], F32, tag="biaso")
            dma(biaso, bo[:].rearrange("(t p) -> p t", p=P))
            # v bias as a single-partition row (free-axis broadcast via matmul)
            bvrow = consts.tile([1, HL], BF16, tag="bvrow")
            dma(bvrow, bv[:, :])

            # SBUF residents: qT/kT [hd, head, tok], v natural [tok, head*hd]
            qT = qkpool.tile([P, NH, L], BF16, tag="qT")
            kT = qkpool.tile([P, NH, L], BF16, tag="kT")
            vN = vpool.tile([P, TT, HL], BF16, tag="vN")

            # DRAM bounce buffers for the 8-core AllGather of attn outputs
            ao_in = dpool.tile([HL, L], BF16, tag="ao_in")
            ag_out = dpool.tile([NCORES, HL, L], BF16, tag="ag_out",
                                addr_space="Shared")

            # =============== Phase A: projections ===========================
            with (
                tc.tile_pool(name="xT", bufs=1) as xTpool,
                tc.tile_pool(name="wstr", bufs=3) as wpool,
                tc.tile_pool(name="wvstr", bufs=2) as wvpool,
                tc.tile_pool(name="psA", bufs=4, space="PSUM") as psA,
                tc.tile_pool(name="psV", bufs=4, space="PSUM") as psV,
            ):
                xT = xTpool.tile([P, NT, L], BF16, tag="xT")
                for ti in range(NT):
                    dma(xT[:, ti, :], xt[ti * P:(ti + 1) * P, :])

                # q/k projections (transposed layout: [dout, tok])
                for wt_d, bias_t, dest in ((wq, biasq, qT), (wk, biask, kT)):
                    for dd in range(NH):
                        w_sb = wpool.tile([P, NT, P], BF16, tag="wstr")
                        dma(w_sb, wt_d[:, dd * P:(dd + 1) * P]
                            .rearrange("(n p) f -> p n f", p=P))
                        for c0 in range(0, L, CH):
                            ps = psA.tile([P, CH], F32, tag="psA")
                            for ki in range(NT):
                                nc.tensor.matmul(ps, w_sb[:, ki, :],
                                                 xT[:, ki, c0:c0 + CH],
                                                 start=(ki == 0), stop=(ki == NT - 1))
                            nc.vector.tensor_scalar_add(
                                dest[:, dd, c0:c0 + CH], ps, bias_t[:, dd:dd + 1])

                # v projection (natural layout: [tok, dout]), bias via rank-1 mm
                for oc in range(0, HL, VC):
                    wv_sb = wvpool.tile([P, NT, VC], BF16, tag="wvstr")
                    dma(wv_sb, wv[:, oc:oc + VC].rearrange("(n p) f -> p n f", p=P))
                    for tt in range(TT):
                        ps = psV.tile([P, VC], F32, tag="psV")
                        for ki in range(NT):
                            nc.tensor.matmul(ps, xT[:, ki, tt * P:(tt + 1) * P],
                                             wv_sb[:, ki, :],
                                             start=(ki == 0), stop=False)
                        nc.tensor.matmul(ps, ones_b[:, :], bvrow[:, oc:oc + VC],
                                         start=False, stop=True)
                        nc.vector.tensor_copy(out=vN[:, tt, oc:oc + VC], in_=ps)

            # =============== Phase B: attention =============================
            with (
                tc.tile_pool(name="ao_sb", bufs=1) as aopool,
                tc.tile_pool(name="ex", bufs=2) as expool,
                tc.tile_pool(name="tr1", bufs=2) as tr1pool,
                tc.tile_pool(name="tr2", bufs=2) as tr2pool,
                tc.tile_pool(name="tr4", bufs=2) as tr4pool,
                tc.tile_pool(name="rden", bufs=2) as rdenpool,
                tc.tile_pool(name="psS", bufs=2, space="PSUM") as psS,
                tc.tile_pool(name="psO", bufs=2, space="PSUM") as psO,
                tc.tile_pool(name="psD", bufs=2, space="PSUM") as psD,
            ):
                ao = aopool.tile([P, NH, L], BF16, tag="ao")
                for h in range(NH):
                    for c0 in range(0, L, CH):
                        ex = expool.tile([P, TT, CH], BF16, tag="ex")
                        for j in range(TT // 2):
                            pss = psS.tile([P, 2, CH], F32, tag="psS")
                            for u in range(2):
                                kt = 2 * j + u
                                nc.tensor.matmul(pss[:, u, :],
                                                 kT[:, h, kt * P:(kt + 1) * P],
                                                 qT[:, h, c0:c0 + CH],
                                                 start=True, stop=True)
                            nc.scalar.activation(ex[:, 2 * j:2 * j + 2, :], pss,
                                                 AF.Exp, scale=SCALE)
                        # denominator: DVE pairwise tree over key tiles
                        t1 = tr1pool.tile([P, 8, CH], BF16, tag="tr1")
                        nc.vector.tensor_add(t1, ex[:, 0:8, :], ex[:, 8:16, :])
                        t2 = tr2pool.tile([P, 4, CH], BF16, tag="tr2")
                        nc.vector.tensor_add(t2, t1[:, 0:4, :], t1[:, 4:8, :])
                        t4 = tr4pool.tile([P, 2, CH], BF16, tag="tr4")
                        nc.vector.tensor_add(t4, t2[:, 0:2, :], t2[:, 2:4, :])
                        t5 = tr4pool.tile([P, CH], BF16, tag="tr5")
                        nc.vector.tensor_add(t5, t4[:, 0, :], t4[:, 1, :])
                        # partition sum -> [1, CH]
                        psd = psD.tile([P, CH], F32, tag="psD")
                        nc.tensor.matmul(psd[0:1, :], ones_c, t5,
                                         start=True, stop=True)
                        den = rdenpool.tile([1, CH], F32, tag="den")
                        nc.vector.tensor_copy(out=r(den), in_=psd[0:1, :])
                        # broadcast denom to all partitions, then reciprocal
                        psb = psD.tile([P, CH], F32, tag="psD")
                        nc.tensor.matmul(psb, r(ones_f[0:1, :]), r(den),
                                         start=True, stop=True)
                        rb = rdenpool.tile([P, CH], F32, tag="rb")
                        nc.vector.reciprocal(out=rb, in_=psb)
                        # attn @ v
                        pso = psO.tile([P, CH], F32, tag="psO")
                        for kt in range(TT):
                            nc.tensor.matmul(pso, vN[:, kt, h * P:(h + 1) * P],
                                             ex[:, kt, :],
                                             start=(kt == 0), stop=(kt == TT - 1))
                        nc.vector.tensor_mul(ao[:, h, c0:c0 + CH], pso, rb)
                    # ship this head's output to the AllGather bounce buffer
                    dma(ao_in[h * P:(h + 1) * P, :], ao[:, h, :])

            # =============== Phase C: AllGather + O projection ==============
            nc.gpsimd.collective_compute(
                "AllGather",
                mybir.AluOpType.bypass,
                replica_groups=[list(range(NCORES))],
                ins=[ao_in[:].opt()],
                outs=[ag_out[:].opt()],
            )

            with (
                tc.tile_pool(name="wo_sbp", bufs=1) as wosbpool,
                tc.tile_pool(name="ao2", bufs=2) as ao2pool,
                tc.tile_pool(name="ost", bufs=3) as ostpool,
                tc.tile_pool(name="psC", bufs=4, space="PSUM") as psC,
            ):
                # wo slice for this core's 256 output dims, fully resident
                wo_sb = wosbpool.tile([P, NT, OD], BF16, tag="wo_sbp")
                dma(wo_sb, wo[:, :].rearrange("(n p) f -> p n f", p=P))

                # each core computes its OD output dims for all B*L tokens
                for bb in range(B):
                    for tc0 in range(0, L, CH):
                        ao2 = ao2pool.tile([P, NT, CH], BF16, tag="ao2")
                        for gi in range(NT):
                            dma(ao2[:, gi, :],
                                ag_out[2 * bb + gi // NH,
                                       (gi % NH) * P:((gi % NH) + 1) * P,
                                       tc0:tc0 + CH])
                        for dd in range(ODT):
                            ps = psC.tile([P, CH], F32, tag="psC")
                            for ki in range(NT):
                                nc.tensor.matmul(ps, wo_sb[:, ki, dd * P:(dd + 1) * P],
                                                 ao2[:, ki, :],
                                                 start=(ki == 0), stop=(ki == NT - 1))
                            o_sb = ostpool.tile([P, CH], F32, tag="ost")
                            nc.vector.tensor_scalar_add(o_sb, ps, biaso[:, dd:dd + 1])
                            dma(yt[dd * P:(dd + 1) * P,
                                   bb * L + tc0:bb * L + tc0 + CH], o_sb)

    nc.compile()
    return nc


def kernel(**inputs):
    inp = {k: np.asarray(v, dtype=np.float32) for k, v in inputs.items()}
    x = inp["x"]

    if "nc" not in _cache:
        _cache["nc"] = _build()
    nc = _cache["nc"]

    # fold LoRA into the dense weights: W' = W + SCALING * A @ B  (exact)
    wT = {}
    for p in "qkvo":
        Wp = inp[f"W{p}"] + SCALING * (inp[f"A{p}"] @ inp[f"B{p}"])
        wT[p] = np.ascontiguousarray(Wp.T)  # [din, dout] fp32

    in_maps = []
    for c in range(NCORES):
        b, hh = c // 2, c % 2
        S = slice(hh * HL, (hh + 1) * HL)
        SO = slice(c * OD, (c + 1) * OD)
        m = {
            "xt": np.ascontiguousarray(x[b].T).astype(BF),
            "wq": np.ascontiguousarray(wT["q"][:, S]).astype(BF),
            "wk": np.ascontiguousarray(wT["k"][:, S]).astype(BF),
            "wv": np.ascontiguousarray(wT["v"][:, S]).astype(BF),
            "wo": np.ascontiguousarray(wT["o"][:, SO]).astype(BF),
            "bq": np.ascontiguousarray(inp["bq"][S]),
            "bk": np.ascontiguousarray(inp["bk"][S]),
            "bv": np.ascontiguousarray(inp["bv"][S]).astype(BF).reshape(1, HL),
            "bo": np.ascontiguousarray(inp["bo"][SO]),
        }
        in_maps.append(m)

    trace = bool(int(os.environ.get("KERNEL_TRACE", "0")))
    res = run_bass_kernel_spmd(nc, in_maps, list(range(NCORES)), trace=trace)
    _cache["last_exec_time_ns"] = res.exec_time_ns
    _cache["last_result"] = res

    y = np.empty((B, L, D), dtype=np.float32)
    for c in range(NCORES):
        yt_c = res.results[c]["yt"]  # [OD, B*L]
        y[:, :, c * OD:(c + 1) * OD] = (
            yt_c.reshape(OD, B, L).transpose(1, 2, 0))
    return y


# revision 17
# speedup vs baseline: 1.0725x; 1.0725x over previous
"""Multi-head self-attention with LoRA on 8 Trainium2 NeuronCores.

Sharding: core c -> (batch b = c//2, head-half hh = c%2).
LoRA is folded into the weights on the host (W' = W + 0.5*A@B, exact).
Each core:
  - projects its 8 heads' q/k (transposed layout) and v (natural layout)
    for all 2048 tokens of its batch, in bf16 (fp32 PSUM accumulation)
  - attention for its 8 heads over all 2048 queries; av matmuls are
    interleaved with the score matmuls (chasing the scalar-engine exp),
    softmax denominators via DVE pairwise tree + one ones-matmul, flushed
    with a one-unit delay so the PE queue never blocks on the DVE
  - two 8-core AllGathers (heads 0-3, 4-7) share attention outputs; the
    first overlaps the second half of attention, the second overlaps the
    first half of each O-projection contraction
  - O-projection resharded by output dim: each core computes its 256
    output dims for all 8192 (batch, token) pairs, written transposed
Host: weight folding/pre-tiling/bf16 casts and output assembly.
"""

import os
import numpy as np
import ml_dtypes

import concourse.bacc as bacc
import concourse.mybir as mybir
import concourse.tile as tile
from concourse.bass_utils import run_bass_kernel_spmd

F32 = mybir.dt.float32
F32R = mybir.dt.float32r
BF16 = mybir.dt.bfloat16
AF = mybir.ActivationFunctionType
BF = ml_dtypes.bfloat16

B, L, D = 4, 2048, 2048
H, HD = 16, 128
SCALING = 0.5          # lora alpha / rank
SCALE = HD ** -0.5     # attention score scale
P = 128                # partitions
NT = D // P            # 16 tiles along the full feature dim
HL = D // 2            # 1024: head-half feature dim per core
NH = 8                 # heads per core
HH = 4                 # heads per AllGather half
TT = L // P            # 16 tiles along token dim
CH = 512               # moving-dim chunk (queries / tokens)
VC = 256               # v-projection dout chunk
NCORES = 8
OD = D // NCORES       # 256: O-projection output dims per core
ODT = OD // P          # 2 output-dim tiles per core

_cache = {}


def _build():
    nc = bacc.Bacc(num_devices=NCORES)

    xt = nc.dram_tensor("xt", [D, L], BF16, kind="ExternalInput")
    # weights pre-tiled on host for contiguous DMA
    wq = nc.dram_tensor("wq", [NH, P, NT, P], BF16, kind="ExternalInput")
    wk = nc.dram_tensor("wk", [NH, P, NT, P], BF16, kind="ExternalInput")
    wv = nc.dram_tensor("wv", [HL // VC, P, NT, VC], BF16, kind="ExternalInput")
    wo = nc.dram_tensor("wo", [P, NT, OD], BF16, kind="ExternalInput")
    bq = nc.dram_tensor("bq", [HL], F32, kind="ExternalInput")
    bk = nc.dram_tensor("bk", [HL], F32, kind="ExternalInput")
    bv = nc.dram_tensor("bv", [1, HL], BF16, kind="ExternalInput")
    bo = nc.dram_tensor("bo", [OD], F32, kind="ExternalInput")
    yt = nc.dram_tensor("yt", [OD, B * L], F32, kind="ExternalOutput")

    ones_f_d = nc.inline_tensor(np.ones((P, P), dtype=np.float32), name="ones_f_d")
    ones_b_d = nc.inline_tensor(np.ones((1, P), dtype=BF), name="ones_b_d")
    ones_c_d = nc.inline_tensor(np.ones((P, 1), dtype=BF), name="ones_c_d")

    def dma(out, in_):
        nc.sync.dma_start(out=out, in_=in_)

    def r(ap):
        return ap.bitcast(F32R)

    with tile.TileContext(nc) as tc:
        with (
            tc.tile_pool(name="consts", bufs=1) as consts,
            tc.tile_pool(name="qk_sb", bufs=1) as qkpool,
            tc.tile_pool(name="v_sb", bufs=1) as vpool,
            tc.tile_pool(name="dram", bufs=1, space="DRAM") as dpool,
        ):
            # ---- persistent constants ----
            ones_f = consts.tile([P, P], F32, tag="ones_f")
            nc.sync.dma_start(out=ones_f.bitcast(F32R),
                              in_=ones_f_d[:, :].bitcast(F32R))
            ones_b = consts.tile([1, P], BF16, tag="ones_b")
            dma(ones_b, ones_b_d[:, :])
            ones_c = consts.tile([P, 1], BF16, tag="ones_c")
            dma(ones_c, ones_c_d[:, :])
            biasq = consts.tile([P, NH], F32, tag="biasq")
            dma(biasq, bq[:].rearrange("(t p) -> p t", p=P))
            biask = consts.tile([P, NH], F32, tag="biask")
            dma(biask, bk[:].rearrange("(t p) -> p t", p=P))
            biaso = consts.tile([P, ODT], F32, tag="biaso")
            dma(biaso, bo[:].rearrange("(t p) -> p t", p=P))
            bvrow = consts.tile([1, HL], BF16, tag="bvrow")
            dma(bvrow, bv[:, :])

            # SBUF residents: qT/kT [hd, head, tok], v natural [tok, head*hd]
            qT = qkpool.tile([P, NH, L], BF16, tag="qT")
            kT = qkpool.tile([P, NH, L], BF16, tag="kT")
            vN = vpool.tile([P, TT, HL], BF16, tag="vN")

            # DRAM bounce buffers for the two 8-core AllGathers
            ao_in1 = dpool.tile([HH * P, L], BF16, tag="ao_in1")
            ao_in2 = dpool.tile([HH * P, L], BF16, tag="ao_in2")
            ag1 = dpool.tile([NCORES, HH * P, L], BF16, tag="ag1",
                             addr_space="Shared")
            ag2 = dpool.tile([NCORES, HH * P, L], BF16, tag="ag2",
                             addr_space="Shared")

            # =============== Phase A: projections ===========================
            with (
                tc.tile_pool(name="xT", bufs=1) as xTpool,
                tc.tile_pool(name="wstr", bufs=3) as wpool,
            ):
                # first q-weight tile before the bulk xT load, so the PE can
                # start as soon as xT tile 0 lands
                w_first = wpool.tile([P, NT, P], BF16, tag="wstr")
                dma(w_first, wq[0])

                xT = xTpool.tile([P, NT, L], BF16, tag="xT")
                for ti in range(NT):
                    dma(xT[:, ti, :], xt[ti * P:(ti + 1) * P, :])

                with tc.tile_pool(name="psA", bufs=4, space="PSUM") as psA:
                    for wt_d, bias_t, dest in ((wq, biasq, qT), (wk, biask, kT)):
                        for dd in range(NH):
                            if wt_d is wq and dd == 0:
                                w_sb = w_first
                            else:
                                w_sb = wpool.tile([P, NT, P], BF16, tag="wstr")
                                dma(w_sb, wt_d[dd])
                            for c0 in range(0, L, CH):
                                ps = psA.tile([P, CH], F32, tag="psA")
                                for ki in range(NT):
                                    nc.tensor.matmul(ps, w_sb[:, ki, :],
                                                     xT[:, ki, c0:c0 + CH],
                                                     start=(ki == 0),
                                                     stop=(ki == NT - 1))
                                nc.vector.tensor_scalar_add(
                                    dest[:, dd, c0:c0 + CH], ps,
                                    bias_t[:, dd:dd + 1])

                # v projection (natural layout), bias via rank-1 matmul
                with (
                    tc.tile_pool(name="wvstr", bufs=2) as wvpool,
                    tc.tile_pool(name="psV", bufs=6, space="PSUM") as psV,
                ):
                    for oc in range(HL // VC):
                        wv_sb = wvpool.tile([P, NT, VC], BF16, tag="wvstr")
                        dma(wv_sb, wv[oc])
                        for tt in range(TT):
                            ps = psV.tile([P, VC], F32, tag="psV")
                            for ki in range(NT):
                                nc.tensor.matmul(ps, xT[:, ki, tt * P:(tt + 1) * P],
                                                 wv_sb[:, ki, :],
                                                 start=(ki == 0), stop=False)
                            nc.tensor.matmul(ps, ones_b[:, :],
                                             bvrow[:, oc * VC:(oc + 1) * VC],
                                             start=False, stop=True)
                            nc.vector.tensor_copy(
                                out=vN[:, tt, oc * VC:(oc + 1) * VC], in_=ps)

            # =============== Phase B: attention =============================
            with (
                tc.tile_pool(name="ao_sb", bufs=1) as aopool,
                tc.tile_pool(name="ex", bufs=2) as expool,
                tc.tile_pool(name="tr1", bufs=2) as tr1pool,
                tc.tile_pool(name="tr2", bufs=2) as tr2pool,
                tc.tile_pool(name="tr4", bufs=2) as tr4pool,
                tc.tile_pool(name="rden", bufs=2) as rdenpool,
                tc.tile_pool(name="psS", bufs=2, space="PSUM") as psS,
                tc.tile_pool(name="psO", bufs=2, space="PSUM") as psO,
                tc.tile_pool(name="psD", bufs=2, space="PSUM") as psD,
            ):
                ao = aopool.tile([P, NH, L], BF16, tag="ao")

                def flush(pend):
                    """Denominator + normalize for a finished unit (delayed
                    one unit so the PE never waits on the DVE tree)."""
                    t5, pso, h, c0 = pend
                    psd = psD.tile([P, CH], F32, tag="psD")
                    nc.tensor.matmul(psd[0:1, :], ones_c, t5,
                                     start=True, stop=True)
                    den = rdenpool.tile([1, CH], F32, tag="den")
                    nc.vector.tensor_copy(out=r(den), in_=psd[0:1, :])
                    psb = psD.tile([P, CH], F32, tag="psD")
                    nc.tensor.matmul(psb, r(ones_f[0:1, :]), r(den),
                                     start=True, stop=True)
                    rb = rdenpool.tile([P, CH], F32, tag="rb")
                    nc.vector.reciprocal(out=rb, in_=psb)
                    nc.vector.tensor_mul(ao[:, h, c0:c0 + CH], pso, rb)
                    # ship normalized slice to its AllGather bounce buffer
                    if h < HH:
                        dma(ao_in1[h * P:(h + 1) * P, c0:c0 + CH],
                            ao[:, h, c0:c0 + CH])
                    else:
                        dma(ao_in2[(h - HH) * P:(h - HH + 1) * P, c0:c0 + CH],
                            ao[:, h, c0:c0 + CH])

                pending = None
                for h in range(NH):
                    for c0 in range(0, L, CH):
                        ex = expool.tile([P, TT, CH], BF16, tag="ex")
                        pso = psO.tile([P, CH], F32, tag="psO")
                        # scores/exp pair j interleaved with av of pair j-1
                        for j in range(TT // 2):
                            pss = psS.tile([P, 2, CH], F32, tag="psS")
                            for u in range(2):
                                kt = 2 * j + u
                                nc.tensor.matmul(pss[:, u, :],
                                                 kT[:, h, kt * P:(kt + 1) * P],
                                                 qT[:, h, c0:c0 + CH],
                                                 start=True, stop=True)
                            nc.scalar.activation(ex[:, 2 * j:2 * j + 2, :], pss,
                                                 AF.Exp, scale=SCALE)
                            if j > 0:
                                for kt in (2 * j - 2, 2 * j - 1):
                                    nc.tensor.matmul(
                                        pso, vN[:, kt, h * P:(h + 1) * P],
                                        ex[:, kt, :],
                                        start=(kt == 0), stop=False)
                        for kt in (TT - 2, TT - 1):
                            nc.tensor.matmul(pso, vN[:, kt, h * P:(h + 1) * P],
                                             ex[:, kt, :],
                                             start=False, stop=(kt == TT - 1))
                        # denominator tree on the DVE (runs behind the PE)
                        t1 = tr1pool.tile([P, 8, CH], BF16, tag="tr1")
                        nc.vector.tensor_add(t1, ex[:, 0:8, :], ex[:, 8:16, :])
                        t2 = tr2pool.tile([P, 4, CH], BF16, tag="tr2")
                        nc.vector.tensor_add(t2, t1[:, 0:4, :], t1[:, 4:8, :])
                        t4 = tr4pool.tile([P, 2, CH], BF16, tag="tr4")
                        nc.vector.tensor_add(t4, t2[:, 0:2, :], t2[:, 2:4, :])
                        t5 = tr4pool.tile([P, CH], BF16, tag="tr5")
                        nc.vector.tensor_add(t5, t4[:, 0, :], t4[:, 1, :])

                        if pending is not None:
                            flush(pending)
                        pending = (t5, pso, h, c0)
                    if h == HH - 1:
                        # heads 0-2 fully flushed; flush (3, c0=3) now so
                        # AG1 covers all of ao_in1, then launch it
                        flush(pending)
                        pending = None
                        nc.gpsimd.collective_compute(
                            "AllGather", mybir.AluOpType.bypass,
                            replica_groups=[list(range(NCORES))],
                            ins=[ao_in1[:].opt()], outs=[ag1[:].opt()])
                flush(pending)
                nc.gpsimd.collective_compute(
                    "AllGather", mybir.AluOpType.bypass,
                    replica_groups=[list(range(NCORES))],
                    ins=[ao_in2[:].opt()], outs=[ag2[:].opt()])

            # =============== Phase C: O projection ==========================
            # contraction order: AG1-covered din tiles first, then AG2's
            kis1 = [g for g in range(NT) if g % NH < HH]
            kis2 = [g for g in range(NT) if g % NH >= HH]

            with (
                tc.tile_pool(name="wo_sbp", bufs=1) as wosbpool,
                tc.tile_pool(name="ao2", bufs=3) as ao2pool,
                tc.tile_pool(name="ost", bufs=3) as ostpool,
                tc.tile_pool(name="psC", bufs=4, space="PSUM") as psC,
            ):
                wo_sb = wosbpool.tile([P, NT, OD], BF16, tag="wo_sbp")
                dma(wo_sb, wo[:, :, :])

                for bb in range(B):
                    for tc0 in range(0, L, CH):
                        ao2 = ao2pool.tile([P, NT, CH], BF16, tag="ao2")
                        for gi in kis1:
                            dma(ao2[:, gi, :],
                                ag1[2 * bb + gi // NH,
                                    (gi % NH) * P:(gi % NH + 1) * P,
                                    tc0:tc0 + CH])
                        for gi in kis2:
                            dma(ao2[:, gi, :],
                                ag2[2 * bb + gi // NH,
                                    (gi % NH - HH) * P:(gi % NH - HH + 1) * P,
                                    tc0:tc0 + CH])
                        for dd in range(ODT):
                            ps = psC.tile([P, CH], F32, tag="psC")
                            for i, gi in enumerate(kis1 + kis2):
                                nc.tensor.matmul(ps, wo_sb[:, gi, dd * P:(dd + 1) * P],
                                                 ao2[:, gi, :],
                                                 start=(i == 0), stop=(i == NT - 1))
                            o_sb = ostpool.tile([P, CH], F32, tag="ost")
                            nc.vector.tensor_scalar_add(o_sb, ps, biaso[:, dd:dd + 1])
                            dma(yt[dd * P:(dd + 1) * P,
                                   bb * L + tc0:bb * L + tc0 + CH], o_sb)

    nc.compile()
    return nc


def kernel(**inputs):
    inp = {k: np.asarray(v, dtype=np.float32) for k, v in inputs.items()}
    x = inp["x"]

    if "nc" not in _cache:
        _cache["nc"] = _build()
    nc = _cache["nc"]

    # fold LoRA into the dense weights: W' = W + SCALING * A @ B  (exact)
    wT = {}
    for p in "qkvo":
        Wp = inp[f"W{p}"] + SCALING * (inp[f"A{p}"] @ inp[f"B{p}"])
        wT[p] = np.ascontiguousarray(Wp.T)  # [din, dout] fp32

    def tile_qk(w):  # [D, HL] -> [NH, P, NT, P]
        return np.ascontiguousarray(
            w.reshape(NT, P, NH, P).transpose(2, 1, 0, 3)).astype(BF)

    def tile_v(w):  # [D, HL] -> [HL//VC, P, NT, VC]
        return np.ascontiguousarray(
            w.reshape(NT, P, HL // VC, VC).transpose(2, 1, 0, 3)).astype(BF)

    def tile_o(w):  # [D, OD] -> [P, NT, OD]
        return np.ascontiguousarray(
            w.reshape(NT, P, OD).transpose(1, 0, 2)).astype(BF)

    in_maps = []
    for c in range(NCORES):
        b, hh = c // 2, c % 2
        S = slice(hh * HL, (hh + 1) * HL)
        SO = slice(c * OD, (c + 1) * OD)
        m = {
            "xt": np.ascontiguousarray(x[b].T).astype(BF),
            "wq": tile_qk(wT["q"][:, S]),
            "wk": tile_qk(wT["k"][:, S]),
            "wv": tile_v(wT["v"][:, S]),
            "wo": tile_o(wT["o"][:, SO]),
            "bq": np.ascontiguousarray(inp["bq"][S]),
            "bk": np.ascontiguousarray(inp["bk"][S]),
            "bv": np.ascontiguousarray(inp["bv"][S]).astype(BF).reshape(1, HL),
            "bo": np.ascontiguousarray(inp["bo"][SO]),
        }
        in_maps.append(m)

    trace = bool(int(os.environ.get("KERNEL_TRACE", "0")))
    res = run_bass_kernel_spmd(nc, in_maps, list(range(NCORES)), trace=trace)
    _cache["last_exec_time_ns"] = res.exec_time_ns
    _cache["last_result"] = res

    y = np.empty((B, L, D), dtype=np.float32)
    for c in range(NCORES):
        yt_c = res.results[c]["yt"]  # [OD, B*L]
        y[:, :, c * OD:(c + 1) * OD] = (
            yt_c.reshape(OD, B, L).transpose(1, 2, 0))
    return y


# revision 24
# speedup vs baseline: 1.1440x; 1.0667x over previous
"""Multi-head self-attention with LoRA on 8 Trainium2 NeuronCores.

Sharding: core c -> (batch b = c//2, head-half hh = c%2).
LoRA is folded into the weights on the host (W' = W + 0.5*A@B, exact).
Each core:
  - projects its 8 heads' q/k (transposed layout) and v (natural layout)
    for all 2048 tokens of its batch, in bf16 (fp32 PSUM accumulation)
  - attention for its 8 heads over all 2048 queries; av matmuls are
    interleaved with the score matmuls (chasing the scalar-engine exp),
    softmax denominators via DVE pairwise tree + one ones-matmul, flushed
    with a one-unit delay so the PE queue never blocks on the DVE
  - two 8-core AllGathers (heads 0-3, 4-7) share attention outputs; the
    first overlaps the second half of attention, the second overlaps the
    first half of each O-projection contraction
  - O-projection resharded by output dim: each core computes its 256
    output dims for all 8192 (batch, token) pairs, written transposed
Host: weight folding/pre-tiling/bf16 casts and output assembly.
"""

import os
import numpy as np
import ml_dtypes

import concourse.bacc as bacc
import concourse.mybir as mybir
import concourse.tile as tile
from concourse.bass_utils import run_bass_kernel_spmd

F32 = mybir.dt.float32
F32R = mybir.dt.float32r
BF16 = mybir.dt.bfloat16
AF = mybir.ActivationFunctionType
BF = ml_dtypes.bfloat16

B, L, D = 4, 2048, 2048
H, HD = 16, 128
SCALING = 0.5          # lora alpha / rank
SCALE = HD ** -0.5     # attention score scale
P = 128                # partitions
NT = D // P            # 16 tiles along the full feature dim
HL = D // 2            # 1024: head-half feature dim per core
NH = 8                 # heads per core
HH = 4                 # heads per AllGather half
TT = L // P            # 16 tiles along token dim
CH = 512               # moving-dim chunk (queries / tokens)
VC = 256               # v-projection dout chunk
NCORES = 8
OD = D // NCORES       # 256: O-projection output dims per core
ODT = OD // P          # 2 output-dim tiles per core

AG_GROUPS = [[0, 1], [2, 3], [4, 5], [6], [7]]  # heads per AllGather stage

_cache = {}


def _build():
    nc = bacc.Bacc(num_devices=NCORES)

    xt = nc.dram_tensor("xt", [D, L], BF16, kind="ExternalInput")
    # weights pre-tiled on host for contiguous DMA
    wq = nc.dram_tensor("wq", [NH, P, NT, P], BF16, kind="ExternalInput")
    wk = nc.dram_tensor("wk", [NH, P, NT, P], BF16, kind="ExternalInput")
    wv = nc.dram_tensor("wv", [P, NT, HL], BF16, kind="ExternalInput")
    wo = nc.dram_tensor("wo", [P, NT, OD], BF16, kind="ExternalInput")
    bq = nc.dram_tensor("bq", [HL], F32, kind="ExternalInput")
    bk = nc.dram_tensor("bk", [HL], F32, kind="ExternalInput")
    bv = nc.dram_tensor("bv", [1, HL], BF16, kind="ExternalInput")
    bo = nc.dram_tensor("bo", [OD], F32, kind="ExternalInput")
    yt = nc.dram_tensor("yt", [OD, B * L], F32, kind="ExternalOutput")

    ones_f_d = nc.inline_tensor(np.ones((P, P), dtype=np.float32), name="ones_f_d")
    ones_b_d = nc.inline_tensor(np.ones((1, P), dtype=BF), name="ones_b_d")
    ones_c_d = nc.inline_tensor(np.ones((P, 1), dtype=BF), name="ones_c_d")

    def dma(out, in_):
        nc.sync.dma_start(out=out, in_=in_)

    def r(ap):
        return ap.bitcast(F32R)

    with tile.TileContext(nc) as tc:
        with (
            tc.tile_pool(name="consts", bufs=1) as consts,
            tc.tile_pool(name="qk_sb", bufs=1) as qkpool,
            tc.tile_pool(name="v_sb", bufs=1) as vpool,
            tc.tile_pool(name="dram", bufs=1, space="DRAM") as dpool,
        ):
            # ---- persistent constants ----
            ones_f = consts.tile([P, P], F32, tag="ones_f")
            nc.sync.dma_start(out=ones_f.bitcast(F32R),
                              in_=ones_f_d[:, :].bitcast(F32R))
            ones_b = consts.tile([1, P], BF16, tag="ones_b")
            dma(ones_b, ones_b_d[:, :])
            ones_c = consts.tile([P, 1], BF16, tag="ones_c")
            dma(ones_c, ones_c_d[:, :])
            biasq = consts.tile([P, NH], F32, tag="biasq")
            dma(biasq, bq[:].rearrange("(t p) -> p t", p=P))
            biask = consts.tile([P, NH], F32, tag="biask")
            dma(biask, bk[:].rearrange("(t p) -> p t", p=P))
            biaso = consts.tile([P, ODT], F32, tag="biaso")
            dma(biaso, bo[:].rearrange("(t p) -> p t", p=P))
            bvrow = consts.tile([1, HL], BF16, tag="bvrow")
            dma(bvrow, bv[:, :])

            # SBUF residents: qT/kT [hd, head, tok], v natural [tok, head*hd]
            qT = qkpool.tile([P, NH, L], BF16, tag="qT")
            kT = qkpool.tile([P, NH, L], BF16, tag="kT")
            vN = vpool.tile([P, TT, HL], BF16, tag="vN")

            # DRAM bounce buffers for the staged 8-core AllGathers.
            # Head groups [0,1],[2,3],[4,5],[6],[7]: early groups overlap
            # attention; the small final groups shrink the exposed tail.
            ao_in = dpool.tile([HL, L], BF16, tag="ao_in")
            ags = []
            for g, heads in enumerate(AG_GROUPS):
                t = dpool.tile([NCORES, len(heads) * P, L], BF16,
                               tag=f"ag{g}", name=f"ag{g}",
                               addr_space="Shared")
                ags.append(t)

            # =============== Phase A: projections ===========================
            with (
                tc.tile_pool(name="xT", bufs=1) as xTpool,
                tc.tile_pool(name="wstr", bufs=3) as wpool,
            ):
                # first q-weight tile before the bulk xT load, so the PE can
                # start as soon as xT tile 0 lands
                w_first = wpool.tile([P, NT, P], BF16, tag="wstr")
                dma(w_first, wq[0])

                xT = xTpool.tile([P, NT, L], BF16, tag="xT")
                for ti in range(NT):
                    dma(xT[:, ti, :], xt[ti * P:(ti + 1) * P, :])

                with tc.tile_pool(name="psA", bufs=4, space="PSUM") as psA:
                    for wt_d, bias_t, dest in ((wq, biasq, qT), (wk, biask, kT)):
                        for dd in range(NH):
                            if wt_d is wq and dd == 0:
                                w_sb = w_first
                            else:
                                w_sb = wpool.tile([P, NT, P], BF16, tag="wstr")
                                dma(w_sb, wt_d[dd])
                            for c0 in range(0, L, CH):
                                ps = psA.tile([P, CH], F32, tag="psA")
                                for ki in range(NT):
                                    nc.tensor.matmul(ps, w_sb[:, ki, :],
                                                     xT[:, ki, c0:c0 + CH],
                                                     start=(ki == 0),
                                                     stop=(ki == NT - 1))
                                nc.vector.tensor_scalar_add(
                                    dest[:, dd, c0:c0 + CH], ps,
                                    bias_t[:, dd:dd + 1])

                # v projection (natural layout), bias via rank-1 matmul;
                # the xT stationary is reused across both dout halves
                with (
                    tc.tile_pool(name="wv_sbp", bufs=1) as wvpool,
                    tc.tile_pool(name="psV", bufs=4, space="PSUM") as psV,
                ):
                    wv_sb = wvpool.tile([P, NT, HL], BF16, tag="wv_sbp")
                    dma(wv_sb, wv[:, :, :])
                    for tt in range(TT):
                        ps0 = psV.tile([P, CH], F32, tag="psV")
                        ps1 = psV.tile([P, CH], F32, tag="psV")
                        for ki in range(NT):
                            nc.tensor.matmul(ps0, xT[:, ki, tt * P:(tt + 1) * P],
                                             wv_sb[:, ki, 0:CH],
                                             start=(ki == 0), stop=False)
                            nc.tensor.matmul(ps1, xT[:, ki, tt * P:(tt + 1) * P],
                                             wv_sb[:, ki, CH:HL],
                                             start=(ki == 0), stop=False)
                        for oc, ps in ((0, ps0), (1, ps1)):
                            nc.tensor.matmul(ps, ones_b[:, :],
                                             bvrow[:, oc * CH:(oc + 1) * CH],
                                             start=False, stop=True)
                            nc.vector.tensor_copy(
                                out=vN[:, tt, oc * CH:(oc + 1) * CH], in_=ps)

            # =============== Phase B: attention =============================
            with (
                tc.tile_pool(name="ao_sb", bufs=1) as aopool,
                tc.tile_pool(name="ex", bufs=2) as expool,
                tc.tile_pool(name="tr1", bufs=2) as tr1pool,
                tc.tile_pool(name="tr2", bufs=2) as tr2pool,
                tc.tile_pool(name="tr4", bufs=2) as tr4pool,
                tc.tile_pool(name="rden", bufs=2) as rdenpool,
                tc.tile_pool(name="psS", bufs=2, space="PSUM") as psS,
                tc.tile_pool(name="psO", bufs=2, space="PSUM") as psO,
                tc.tile_pool(name="psD", bufs=2, space="PSUM") as psD,
            ):
                ao = aopool.tile([P, NH, L], BF16, tag="ao")
                last_head_of = {g[-1]: i for i, g in enumerate(AG_GROUPS)}

                def flush(pend):
                    """Denominator + normalize for a finished unit (delayed
                    one unit so the PE never waits on the DVE tree)."""
                    t5, pso, h, c0 = pend
                    psd = psD.tile([P, CH], F32, tag="psD")
                    nc.tensor.matmul(psd[0:1, :], ones_c, t5,
                                     start=True, stop=True)
                    den = rdenpool.tile([1, CH], F32, tag="den")
                    nc.vector.tensor_copy(out=r(den), in_=psd[0:1, :])
                    psb = psD.tile([P, CH], F32, tag="psD")
                    nc.tensor.matmul(psb, r(ones_f[0:1, :]), r(den),
                                     start=True, stop=True)
                    rb = rdenpool.tile([P, CH], F32, tag="rb")
                    nc.vector.reciprocal(out=rb, in_=psb)
                    nc.vector.tensor_mul(ao[:, h, c0:c0 + CH], pso, rb)
                    # ship normalized slice to the AllGather bounce buffer
                    dma(ao_in[h * P:(h + 1) * P, c0:c0 + CH],
                        ao[:, h, c0:c0 + CH])
                    # once a head group's last slice is shipped, gather it
                    if c0 == L - CH and h in last_head_of:
                        g = last_head_of[h]
                        heads = AG_GROUPS[g]
                        nc.gpsimd.collective_compute(
                            "AllGather", mybir.AluOpType.bypass,
                            replica_groups=[list(range(NCORES))],
                            ins=[ao_in[heads[0] * P:
                                       (heads[-1] + 1) * P, :].opt()],
                            outs=[ags[g][:].opt()])

                pending = None
                for h in range(NH):
                    for c0 in range(0, L, CH):
                        ex = expool.tile([P, TT, CH], BF16, tag="ex")
                        pso = psO.tile([P, CH], F32, tag="psO")
                        # scores/exp pair j interleaved with av of pair j-1
                        for j in range(TT // 2):
                            pss = psS.tile([P, 2, CH], F32, tag="psS")
                            for u in range(2):
                                kt = 2 * j + u
                                nc.tensor.matmul(pss[:, u, :],
                                                 kT[:, h, kt * P:(kt + 1) * P],
                                                 qT[:, h, c0:c0 + CH],
                                                 start=True, stop=True)
                            nc.scalar.activation(ex[:, 2 * j:2 * j + 2, :], pss,
                                                 AF.Exp, scale=SCALE)
                            if j > 0:
                                for kt in (2 * j - 2, 2 * j - 1):
                                    nc.tensor.matmul(
                                        pso, vN[:, kt, h * P:(h + 1) * P],
                                        ex[:, kt, :],
                                        start=(kt == 0), stop=False)
                        for kt in (TT - 2, TT - 1):
                            nc.tensor.matmul(pso, vN[:, kt, h * P:(h + 1) * P],
                                             ex[:, kt, :],
                                             start=False, stop=(kt == TT - 1))
                        # denominator tree on the DVE (runs behind the PE)
                        t1 = tr1pool.tile([P, 8, CH], BF16, tag="tr1")
                        nc.vector.tensor_add(t1, ex[:, 0:8, :], ex[:, 8:16, :])
                        t2 = tr2pool.tile([P, 4, CH], BF16, tag="tr2")
                        nc.vector.tensor_add(t2, t1[:, 0:4, :], t1[:, 4:8, :])
                        t4 = tr4pool.tile([P, 2, CH], BF16, tag="tr4")
                        nc.vector.tensor_add(t4, t2[:, 0:2, :], t2[:, 2:4, :])
                        t5 = tr4pool.tile([P, CH], BF16, tag="tr5")
                        nc.vector.tensor_add(t5, t4[:, 0, :], t4[:, 1, :])

                        if pending is not None:
                            flush(pending)
                        pending = (t5, pso, h, c0)
                flush(pending)

            # =============== Phase C: O projection ==========================
            # contraction ordered by AllGather stage so early-stage matmuls
            # run while the last gathers are still in flight
            part = []  # (gi within pair, ag index, row tile within ag)
            for g, heads in enumerate(AG_GROUPS):
                for hi, ht in enumerate(heads):
                    for rr in range(2):
                        part.append((rr * NH + ht, g, hi))

            with (
                tc.tile_pool(name="wo_sbp", bufs=1) as wosbpool,
                tc.tile_pool(name="ao2", bufs=3) as ao2pool,
                tc.tile_pool(name="ost", bufs=3) as ostpool,
                tc.tile_pool(name="psC", bufs=4, space="PSUM") as psC,
            ):
                wo_sb = wosbpool.tile([P, NT, OD], BF16, tag="wo_sbp")
                dma(wo_sb, wo[:, :, :])

                for bb in range(B):
                    for tc0 in range(0, L, CH):
                        ao2 = ao2pool.tile([P, NT, CH], BF16, tag="ao2")
                        for gi, g, hi in part:
                            dma(ao2[:, gi, :],
                                ags[g][2 * bb + gi // NH,
                                       hi * P:(hi + 1) * P, tc0:tc0 + CH])
                        for dd in range(ODT):
                            ps = psC.tile([P, CH], F32, tag="psC")
                            for i, (gi, _, _) in enumerate(part):
                                nc.tensor.matmul(ps, wo_sb[:, gi, dd * P:(dd + 1) * P],
                                                 ao2[:, gi, :],
                                                 start=(i == 0), stop=(i == NT - 1))
                            o_sb = ostpool.tile([P, CH], F32, tag="ost")
                            nc.vector.tensor_scalar_add(o_sb, ps, biaso[:, dd:dd + 1])
                            dma(yt[dd * P:(dd + 1) * P,
                                   bb * L + tc0:bb * L + tc0 + CH], o_sb)

    nc.compile()
    return nc


def kernel(**inputs):
    inp = {k: np.asarray(v, dtype=np.float32) for k, v in inputs.items()}
    x = inp["x"]

    if "nc" not in _cache:
        _cache["nc"] = _build()
    nc = _cache["nc"]

    # fold LoRA into the dense weights: W' = W + SCALING * A @ B  (exact)
    wT = {}
    for p in "qkvo":
        Wp = inp[f"W{p}"] + SCALING * (inp[f"A{p}"] @ inp[f"B{p}"])
        wT[p] = np.ascontiguousarray(Wp.T)  # [din, dout] fp32

    def tile_qk(w):  # [D, HL] -> [NH, P, NT, P]
        return np.ascontiguousarray(
            w.reshape(NT, P, NH, P).transpose(2, 1, 0, 3)).astype(BF)

    def tile_v(w):  # [D, HL] -> [P, NT, HL]
        return np.ascontiguousarray(
            w.reshape(NT, P, HL).transpose(1, 0, 2)).astype(BF)

    def tile_o(w):  # [D, OD] -> [P, NT, OD]
        return np.ascontiguousarray(
            w.reshape(NT, P, OD).transpose(1, 0, 2)).astype(BF)

    in_maps = []
    for c in range(NCORES):
        b, hh = c // 2, c % 2
        S = slice(hh * HL, (hh + 1) * HL)
        SO = slice(c * OD, (c + 1) * OD)
        m = {
            "xt": np.ascontiguousarray(x[b].T).astype(BF),
            "wq": tile_qk(wT["q"][:, S]),
            "wk": tile_qk(wT["k"][:, S]),
            "wv": tile_v(wT["v"][:, S]),
            "wo": tile_o(wT["o"][:, SO]),
            "bq": np.ascontiguousarray(inp["bq"][S]),
            "bk": np.ascontiguousarray(inp["bk"][S]),
            "bv": np.ascontiguousarray(inp["bv"][S]).astype(BF).reshape(1, HL),
            "bo": np.ascontiguousarray(inp["bo"][SO]),
        }
        in_maps.append(m)

    trace = bool(int(os.environ.get("KERNEL_TRACE", "0")))
    res = run_bass_kernel_spmd(nc, in_maps, list(range(NCORES)), trace=trace)
    _cache["last_exec_time_ns"] = res.exec_time_ns
    _cache["last_result"] = res

    y = np.empty((B, L, D), dtype=np.float32)
    for c in range(NCORES):
        yt_c = res.results[c]["yt"]  # [OD, B*L]
        y[:, :, c * OD:(c + 1) * OD] = (
            yt_c.reshape(OD, B, L).transpose(1, 2, 0))
    return y


# revision 26
# speedup vs baseline: 1.2113x; 1.0589x over previous
"""Multi-head self-attention with LoRA on 8 Trainium2 NeuronCores.

Sharding: core c -> (batch b = c//2, head-half hh = c%2).
LoRA is folded into the weights on the host (W' = W + 0.5*A@B, exact).
Each core:
  - projects its 8 heads' q/k (transposed layout) and v (natural layout)
    for all 2048 tokens of its batch, in bf16 (fp32 PSUM accumulation)
  - attention for its 8 heads over all 2048 queries; av matmuls are
    interleaved with the score matmuls (chasing the scalar-engine exp),
    softmax denominators via DVE pairwise tree + one ones-matmul, flushed
    with a one-unit delay so the PE queue never blocks on the DVE; the
    v bias is added during the normalize (softmax weights sum to 1)
  - staged 8-core AllGathers (head groups [2,2,2,1,1]) share attention
    outputs while later heads still compute
  - O-projection for this core's own 1024 tokens over all 2048 output
    dims; the gather blocks for this core's batch are selected with
    partition_id-derived dynamic DMA offsets (4MB staged, not 32MB);
    the first token chunk runs two passes so the PE chews AG-independent
    contraction parts while the final AllGather lands
Host: weight folding/pre-tiling/bf16 casts and output assembly.
"""

import os
import numpy as np
import ml_dtypes

import concourse.bass as bass
import concourse.bacc as bacc
import concourse.mybir as mybir
import concourse.tile as tile
from concourse.bass_utils import run_bass_kernel_spmd

F32 = mybir.dt.float32
F32R = mybir.dt.float32r
BF16 = mybir.dt.bfloat16
AF = mybir.ActivationFunctionType
BF = ml_dtypes.bfloat16

B, L, D = 4, 2048, 2048
H, HD = 16, 128
SCALING = 0.5          # lora alpha / rank
SCALE = HD ** -0.5     # attention score scale
P = 128                # partitions
NT = D // P            # 16 tiles along the full feature dim
HL = D // 2            # 1024: head-half feature dim per core
NH = 8                 # heads per core
TT = L // P            # 16 tiles along token dim
CH = 512               # moving-dim chunk (queries / tokens)
NCORES = 8
LH = L // 2            # 1024: tokens per core in the O projection

AG_GROUPS = [[0, 1], [2, 3], [4, 5], [6], [7]]  # heads per AllGather stage

_cache = {}


def _build():
    nc = bacc.Bacc(num_devices=NCORES)

    xt = nc.dram_tensor("xt", [D, L], BF16, kind="ExternalInput")
    # weights pre-tiled on host for contiguous DMA
    wq = nc.dram_tensor("wq", [NH, P, NT, P], BF16, kind="ExternalInput")
    wk = nc.dram_tensor("wk", [NH, P, NT, P], BF16, kind="ExternalInput")
    wv = nc.dram_tensor("wv", [P, NT, HL], BF16, kind="ExternalInput")
    wo = nc.dram_tensor("wo", [P, NT, D], BF16, kind="ExternalInput")
    bq = nc.dram_tensor("bq", [HL], F32, kind="ExternalInput")
    bk = nc.dram_tensor("bk", [HL], F32, kind="ExternalInput")
    bv = nc.dram_tensor("bv", [HL], F32, kind="ExternalInput")
    bo = nc.dram_tensor("bo", [D], F32, kind="ExternalInput")
    yt = nc.dram_tensor("yt", [D, LH], F32, kind="ExternalOutput")

    ones_f_d = nc.inline_tensor(np.ones((P, P), dtype=np.float32), name="ones_f_d")
    ones_c_d = nc.inline_tensor(np.ones((P, 1), dtype=BF), name="ones_c_d")

    def dma(out, in_):
        nc.sync.dma_start(out=out, in_=in_)

    def r(ap):
        return ap.bitcast(F32R)

    with tile.TileContext(nc) as tc:
        pid = nc.partition_id()
        col0 = (pid % 2) * LH          # this core's token-half offset
        blk0 = (pid // 2) * 2          # first gather block of this batch

        with (
            tc.tile_pool(name="consts", bufs=1) as consts,
            tc.tile_pool(name="dram", bufs=1, space="DRAM") as dpool,
        ):
            # ---- persistent constants ----
            ones_f = consts.tile([P, P], F32, tag="ones_f")
            nc.sync.dma_start(out=ones_f.bitcast(F32R),
                              in_=ones_f_d[:, :].bitcast(F32R))
            ones_c = consts.tile([P, 1], BF16, tag="ones_c")
            dma(ones_c, ones_c_d[:, :])
            biasq = consts.tile([P, NH], F32, tag="biasq")
            dma(biasq, bq[:].rearrange("(t p) -> p t", p=P))
            biask = consts.tile([P, NH], F32, tag="biask")
            dma(biask, bk[:].rearrange("(t p) -> p t", p=P))
            biasv = consts.tile([P, NH], F32, tag="biasv")
            dma(biasv, bv[:].rearrange("(t p) -> p t", p=P))
            biaso = consts.tile([P, NT], F32, tag="biaso")
            dma(biaso, bo[:].rearrange("(t p) -> p t", p=P))

            # DRAM bounce buffers for the staged 8-core AllGathers
            ao_in = dpool.tile([HL, L], BF16, tag="ao_in")
            ags = []
            for g, heads in enumerate(AG_GROUPS):
                t = dpool.tile([NCORES, len(heads) * P, L], BF16,
                               tag=f"ag{g}", name=f"ag{g}",
                               addr_space="Shared")
                ags.append(t)

            with (
                tc.tile_pool(name="qk_sb", bufs=1) as qkpool,
                tc.tile_pool(name="v_sb", bufs=1) as vpool,
            ):
                # SBUF residents: qT/kT [hd, head, tok], v nat [tok, hd*head]
                qT = qkpool.tile([P, NH, L], BF16, tag="qT")
                kT = qkpool.tile([P, NH, L], BF16, tag="kT")
                vN = vpool.tile([P, TT, HL], BF16, tag="vN")

                # =============== Phase A: projections =======================
                with (
                    tc.tile_pool(name="xT", bufs=1) as xTpool,
                    tc.tile_pool(name="wstr", bufs=3) as wpool,
                ):
                    # first q-weight tile before the bulk xT load
                    w_first = wpool.tile([P, NT, P], BF16, tag="wstr")
                    dma(w_first, wq[0])

                    xT = xTpool.tile([P, NT, L], BF16, tag="xT")
                    for ti in range(NT):
                        dma(xT[:, ti, :], xt[ti * P:(ti + 1) * P, :])

                    with tc.tile_pool(name="psA", bufs=4, space="PSUM") as psA:
                        for wt_d, bias_t, dest in ((wq, biasq, qT),
                                                   (wk, biask, kT)):
                            for dd in range(NH):
                                if wt_d is wq and dd == 0:
                                    w_sb = w_first
                                else:
                                    w_sb = wpool.tile([P, NT, P], BF16,
                                                      tag="wstr")
                                    dma(w_sb, wt_d[dd])
                                for c0 in range(0, L, CH):
                                    ps = psA.tile([P, CH], F32, tag="psA")
                                    for ki in range(NT):
                                        nc.tensor.matmul(
                                            ps, w_sb[:, ki, :],
                                            xT[:, ki, c0:c0 + CH],
                                            start=(ki == 0),
                                            stop=(ki == NT - 1))
                                    nc.vector.tensor_scalar_add(
                                        dest[:, dd, c0:c0 + CH], ps,
                                        bias_t[:, dd:dd + 1])

                    # v projection; xT stationary reused across dout halves
                    with (
                        tc.tile_pool(name="wv_sbp", bufs=1) as wvpool,
                        tc.tile_pool(name="psV", bufs=4, space="PSUM") as psV,
                    ):
                        wv_sb = wvpool.tile([P, NT, HL], BF16, tag="wv_sbp")
                        dma(wv_sb, wv[:, :, :])
                        for tt in range(TT):
                            ps0 = psV.tile([P, CH], F32, tag="psV")
                            ps1 = psV.tile([P, CH], F32, tag="psV")
                            for ki in range(NT):
                                nc.tensor.matmul(ps0,
                                                 xT[:, ki, tt * P:(tt + 1) * P],
                                                 wv_sb[:, ki, 0:CH],
                                                 start=(ki == 0),
                                                 stop=(ki == NT - 1))
                                nc.tensor.matmul(ps1,
                                                 xT[:, ki, tt * P:(tt + 1) * P],
                                                 wv_sb[:, ki, CH:HL],
                                                 start=(ki == 0),
                                                 stop=(ki == NT - 1))
                            nc.vector.tensor_copy(out=vN[:, tt, 0:CH], in_=ps0)
                            nc.vector.tensor_copy(out=vN[:, tt, CH:HL], in_=ps1)

                # =============== Phase B: attention =========================
                with (
                    tc.tile_pool(name="ao_sb", bufs=1) as aopool,
                    tc.tile_pool(name="ex", bufs=2) as expool,
                    tc.tile_pool(name="tr1", bufs=2) as tr1pool,
                    tc.tile_pool(name="tr2", bufs=2) as tr2pool,
                    tc.tile_pool(name="tr4", bufs=2) as tr4pool,
                    tc.tile_pool(name="rden", bufs=2) as rdenpool,
                    tc.tile_pool(name="psS", bufs=2, space="PSUM") as psS,
                    tc.tile_pool(name="psO", bufs=2, space="PSUM") as psO,
                    tc.tile_pool(name="psD", bufs=2, space="PSUM") as psD,
                ):
                    ao = aopool.tile([P, NH, L], BF16, tag="ao")
                    last_head_of = {g[-1]: i for i, g in enumerate(AG_GROUPS)}

                    def flush(pend):
                        """Denominator + normalize + v-bias for a finished
                        unit (delayed one unit)."""
                        t5, pso, h, c0 = pend
                        psd = psD.tile([P, CH], F32, tag="psD")
                        nc.tensor.matmul(psd[0:1, :], ones_c, t5,
                                         start=True, stop=True)
                        den = rdenpool.tile([1, CH], F32, tag="den")
                        nc.vector.tensor_copy(out=r(den), in_=psd[0:1, :])
                        psb = psD.tile([P, CH], F32, tag="psD")
                        nc.tensor.matmul(psb, r(ones_f[0:1, :]), r(den),
                                         start=True, stop=True)
                        rb = rdenpool.tile([P, CH], F32, tag="rb")
                        nc.vector.reciprocal(out=rb, in_=psb)
                        tmp = rdenpool.tile([P, CH], F32, tag="aotmp")
                        nc.vector.tensor_mul(tmp, pso, rb)
                        nc.vector.tensor_scalar_add(
                            ao[:, h, c0:c0 + CH], tmp, biasv[:, h:h + 1])
                        dma(ao_in[h * P:(h + 1) * P, c0:c0 + CH],
                            ao[:, h, c0:c0 + CH])
                        if c0 == L - CH and h in last_head_of:
                            g = last_head_of[h]
                            heads = AG_GROUPS[g]
                            nc.gpsimd.collective_compute(
                                "AllGather", mybir.AluOpType.bypass,
                                replica_groups=[list(range(NCORES))],
                                ins=[ao_in[heads[0] * P:
                                           (heads[-1] + 1) * P, :].opt()],
                                outs=[ags[g][:].opt()])

                    pending = None
                    for h in range(NH):
                        for c0 in range(0, L, CH):
                            ex = expool.tile([P, TT, CH], BF16, tag="ex")
                            pso = psO.tile([P, CH], F32, tag="psO")
                            # scores pair j interleaved with av of pair j-2
                            for j in range(TT // 2):
                                pss = psS.tile([P, 2, CH], F32, tag="psS")
                                for u in range(2):
                                    kt = 2 * j + u
                                    nc.tensor.matmul(
                                        pss[:, u, :],
                                        kT[:, h, kt * P:(kt + 1) * P],
                                        qT[:, h, c0:c0 + CH],
                                        start=True, stop=True)
                                nc.scalar.activation(ex[:, 2 * j:2 * j + 2, :],
                                                     pss, AF.Exp, scale=SCALE)
                                if j >= 2:
                                    for kt in (2 * j - 4, 2 * j - 3):
                                        nc.tensor.matmul(
                                            pso, vN[:, kt, h * P:(h + 1) * P],
                                            ex[:, kt, :],
                                            start=(kt == 0), stop=False)
                            for kt in range(TT - 4, TT):
                                nc.tensor.matmul(pso,
                                                 vN[:, kt, h * P:(h + 1) * P],
                                                 ex[:, kt, :],
                                                 start=False,
                                                 stop=(kt == TT - 1))
                            # denominator tree on the DVE
                            t1 = tr1pool.tile([P, 8, CH], BF16, tag="tr1")
                            nc.vector.tensor_add(t1, ex[:, 0:8, :],
                                                 ex[:, 8:16, :])
                            t2 = tr2pool.tile([P, 4, CH], BF16, tag="tr2")
                            nc.vector.tensor_add(t2, t1[:, 0:4, :],
                                                 t1[:, 4:8, :])
                            t4 = tr4pool.tile([P, 2, CH], BF16, tag="tr4")
                            nc.vector.tensor_add(t4, t2[:, 0:2, :],
                                                 t2[:, 2:4, :])
                            t5 = tr4pool.tile([P, CH], BF16, tag="tr5")
                            nc.vector.tensor_add(t5, t4[:, 0, :], t4[:, 1, :])

                            if pending is not None:
                                flush(pending)
                            pending = (t5, pso, h, c0)
                    flush(pending)

            # =============== Phase C: O projection ==========================
            # this core's 1024 tokens x all 2048 output dims; gather blocks
            # chosen via partition-id-derived dynamic DMA offsets
            part = []  # (gi within pair, ag index, row tile within ag)
            for g, heads in enumerate(AG_GROUPS):
                for hi, ht in enumerate(heads):
                    for rr in range(2):
                        part.append((rr * NH + ht, g, hi))
            n_pre = sum(1 for _, g, _ in part if g < len(AG_GROUPS) - 1)

            with (
                tc.tile_pool(name="wo_sbp", bufs=1) as wosbpool,
                tc.tile_pool(name="ao2", bufs=2) as ao2pool,
                tc.tile_pool(name="partial", bufs=1) as partpool,
                tc.tile_pool(name="ost", bufs=3) as ostpool,
                tc.tile_pool(name="psC", bufs=4, space="PSUM") as psC,
            ):
                wo_sb = wosbpool.tile([P, NT, D], BF16, tag="wo_sbp")
                dma(wo_sb, wo[:, :, :])
                partial = partpool.tile([P, NT, CH], F32, tag="partial")

                def stage(tc0):
                    ao2 = ao2pool.tile([P, NT, CH], BF16, tag="ao2")
                    for gi, g, hi in part:
                        rr = gi // NH
                        src = ags[g][bass.ds(blk0 + rr, 1),
                                     hi * P:(hi + 1) * P,
                                     bass.ds(col0 + tc0, CH)]
                        dma(ao2[:, gi:gi + 1, :].transpose([0, 2, 1]),
                            src.transpose([1, 2, 0]))
                    return ao2

                # ---- token chunk 0: two passes to absorb the last AG ----
                ao2 = stage(0)
                for dd in range(NT):
                    ps = psC.tile([P, CH], F32, tag="psC")
                    for i, (gi, g, hi) in enumerate(part[:n_pre]):
                        nc.tensor.matmul(ps, wo_sb[:, gi, dd * P:(dd + 1) * P],
                                         ao2[:, gi, :],
                                         start=(i == 0), stop=(i == n_pre - 1))
                    nc.vector.tensor_copy(out=partial[:, dd, :], in_=ps)
                for dd in range(NT):
                    ps = psC.tile([P, CH], F32, tag="psC")
                    for i, (gi, g, hi) in enumerate(part[n_pre:]):
                        nc.tensor.matmul(ps, wo_sb[:, gi, dd * P:(dd + 1) * P],
                                         ao2[:, gi, :],
                                         start=(i == 0),
                                         stop=(i == NT - n_pre - 1))
                    o_sb = ostpool.tile([P, CH], F32, tag="ost")
                    nc.vector.scalar_tensor_tensor(
                        o_sb, ps, biaso[:, dd:dd + 1], partial[:, dd, :],
                        op0=mybir.AluOpType.add, op1=mybir.AluOpType.add)
                    dma(yt[dd * P:(dd + 1) * P, 0:CH], o_sb)

                # ---- token chunk 1: single pass ----
                ao2 = stage(CH)
                for dd in range(NT):
                    ps = psC.tile([P, CH], F32, tag="psC")
                    for i, (gi, g, hi) in enumerate(part):
                        nc.tensor.matmul(ps, wo_sb[:, gi, dd * P:(dd + 1) * P],
                                         ao2[:, gi, :],
                                         start=(i == 0), stop=(i == NT - 1))
                    o_sb = ostpool.tile([P, CH], F32, tag="ost")
                    nc.vector.tensor_scalar_add(o_sb, ps, biaso[:, dd:dd + 1])
                    dma(yt[dd * P:(dd + 1) * P, CH:LH], o_sb)

    nc.compile()
    return nc


def kernel(**inputs):
    inp = {k: np.asarray(v, dtype=np.float32) for k, v in inputs.items()}
    x = inp["x"]

    if "nc" not in _cache:
        _cache["nc"] = _build()
    nc = _cache["nc"]

    # fold LoRA into the dense weights: W' = W + SCALING * A @ B  (exact)
    wT = {}
    for p in "qkvo":
        Wp = inp[f"W{p}"] + SCALING * (inp[f"A{p}"] @ inp[f"B{p}"])
        wT[p] = np.ascontiguousarray(Wp.T)  # [din, dout] fp32

    def tile_qk(w):  # [D, HL] -> [NH, P, NT, P]
        return np.ascontiguousarray(
            w.reshape(NT, P, NH, P).transpose(2, 1, 0, 3)).astype(BF)

    def tile_v(w):  # [D, HL] -> [P, NT, HL]
        return np.ascontiguousarray(
            w.reshape(NT, P, HL).transpose(1, 0, 2)).astype(BF)

    wo_t = np.ascontiguousarray(
        wT["o"].reshape(NT, P, D).transpose(1, 0, 2)).astype(BF)

    in_maps = []
    for c in range(NCORES):
        b, hh = c // 2, c % 2
        S = slice(hh * HL, (hh + 1) * HL)
        m = {
            "xt": np.ascontiguousarray(x[b].T).astype(BF),
            "wq": tile_qk(wT["q"][:, S]),
            "wk": tile_qk(wT["k"][:, S]),
            "wv": tile_v(wT["v"][:, S]),
            "wo": wo_t,
            "bq": np.ascontiguousarray(inp["bq"][S]),
            "bk": np.ascontiguousarray(inp["bk"][S]),
            "bv": np.ascontiguousarray(inp["bv"][S]),
            "bo": inp["bo"],
        }
        in_maps.append(m)

    trace = bool(int(os.environ.get("KERNEL_TRACE", "0")))
    res = run_bass_kernel_spmd(nc, in_maps, list(range(NCORES)), trace=trace)
    _cache["last_exec_time_ns"] = res.exec_time_ns
    _cache["last_result"] = res

    y = np.empty((B, L, D), dtype=np.float32)
    for c in range(NCORES):
        b, hh = c // 2, c % 2
        y[b, hh * LH:(hh + 1) * LH, :] = res.results[c]["yt"].T
    return y


# revision 30
# speedup vs baseline: 1.2162x; 1.0040x over previous
"""Multi-head self-attention with LoRA on 8 Trainium2 NeuronCores.

Sharding: core c -> (batch b = c//2, head-half hh = c%2).
LoRA is folded into the weights on the host (W' = W + 0.5*A@B, exact).
Each core:
  - projects its 8 heads' q/k (transposed layout) and v (natural layout)
    for all 2048 tokens of its batch, in bf16 (fp32 PSUM accumulation)
  - attention for its 8 heads over all 2048 queries; av matmuls are
    interleaved with the score matmuls (chasing the scalar-engine exp),
    softmax denominators via DVE pairwise tree + one ones-matmul, flushed
    with a one-unit delay so the PE queue never blocks on the DVE; the
    v bias is added during the normalize (softmax weights sum to 1)
  - staged 8-core AllGathers (head groups [2,2,2,1,1]) share attention
    outputs while later heads still compute
  - O-projection for this core's own 1024 tokens over all 2048 output
    dims; the gather blocks for this core's batch are selected with
    partition_id-derived dynamic DMA offsets (4MB staged, not 32MB);
    the first token chunk runs two passes so the PE chews AG-independent
    contraction parts while the final AllGather lands
Host: weight folding/pre-tiling/bf16 casts and output assembly.
"""

import os
import numpy as np
import ml_dtypes

import concourse.bass as bass
import concourse.bacc as bacc
import concourse.mybir as mybir
import concourse.tile as tile
from concourse.bass_utils import run_bass_kernel_spmd

F32 = mybir.dt.float32
F32R = mybir.dt.float32r
BF16 = mybir.dt.bfloat16
AF = mybir.ActivationFunctionType
BF = ml_dtypes.bfloat16

B, L, D = 4, 2048, 2048
H, HD = 16, 128
SCALING = 0.5          # lora alpha / rank
SCALE = HD ** -0.5     # attention score scale
P = 128                # partitions
NT = D // P            # 16 tiles along the full feature dim
HL = D // 2            # 1024: head-half feature dim per core
NH = 8                 # heads per core
TT = L // P            # 16 tiles along token dim
CH = 512               # moving-dim chunk (queries / tokens)
NCORES = 8
LH = L // 2            # 1024: tokens per core in the O projection

AG_GROUPS = [[0, 1], [2, 3], [4, 5], [6, 7]]  # heads per AllGather stage

_cache = {}


def _build():
    nc = bacc.Bacc(num_devices=NCORES)

    xt = nc.dram_tensor("xt", [D, L], BF16, kind="ExternalInput")
    # weights pre-tiled on host for contiguous DMA
    wq = nc.dram_tensor("wq", [NH, P, NT, P], BF16, kind="ExternalInput")
    wk = nc.dram_tensor("wk", [NH, P, NT, P], BF16, kind="ExternalInput")
    wv = nc.dram_tensor("wv", [P, NT, HL], BF16, kind="ExternalInput")
    wo = nc.dram_tensor("wo", [P, NT, D], BF16, kind="ExternalInput")
    bq = nc.dram_tensor("bq", [HL], F32, kind="ExternalInput")
    bk = nc.dram_tensor("bk", [HL], F32, kind="ExternalInput")
    bv = nc.dram_tensor("bv", [HL], F32, kind="ExternalInput")
    bo = nc.dram_tensor("bo", [D], F32, kind="ExternalInput")
    yt = nc.dram_tensor("yt", [D, LH], F32, kind="ExternalOutput")

    ones_f_d = nc.inline_tensor(np.ones((P, P), dtype=np.float32), name="ones_f_d")
    ones_c_d = nc.inline_tensor(np.ones((P, 1), dtype=BF), name="ones_c_d")

    def dma(out, in_):
        nc.sync.dma_start(out=out, in_=in_)

    def r(ap):
        return ap.bitcast(F32R)

    with tile.TileContext(nc) as tc:
        pid = nc.partition_id()
        col0 = (pid % 2) * LH          # this core's token-half offset
        blk0 = (pid // 2) * 2          # first gather block of this batch

        with (
            tc.tile_pool(name="consts", bufs=1) as consts,
            tc.tile_pool(name="dram", bufs=1, space="DRAM") as dpool,
        ):
            # ---- persistent constants ----
            ones_f = consts.tile([P, P], F32, tag="ones_f")
            nc.sync.dma_start(out=ones_f.bitcast(F32R),
                              in_=ones_f_d[:, :].bitcast(F32R))
            ones_c = consts.tile([P, 1], BF16, tag="ones_c")
            dma(ones_c, ones_c_d[:, :])
            biasq = consts.tile([P, NH], F32, tag="biasq")
            dma(biasq, bq[:].rearrange("(t p) -> p t", p=P))
            biask = consts.tile([P, NH], F32, tag="biask")
            dma(biask, bk[:].rearrange("(t p) -> p t", p=P))
            biasv = consts.tile([P, NH], F32, tag="biasv")
            dma(biasv, bv[:].rearrange("(t p) -> p t", p=P))
            biaso = consts.tile([P, NT], F32, tag="biaso")
            dma(biaso, bo[:].rearrange("(t p) -> p t", p=P))

            # DRAM bounce buffers for the staged 8-core AllGathers
            ao_in = dpool.tile([HL, L], BF16, tag="ao_in")
            ags = []
            for g, heads in enumerate(AG_GROUPS):
                t = dpool.tile([NCORES, len(heads) * P, L], BF16,
                               tag=f"ag{g}", name=f"ag{g}",
                               addr_space="Shared")
                ags.append(t)

            with (
                tc.tile_pool(name="qk_sb", bufs=1) as qkpool,
                tc.tile_pool(name="v_sb", bufs=1) as vpool,
            ):
                # SBUF residents: qT/kT [hd, head, tok], v nat [tok, hd*head]
                qT = qkpool.tile([P, NH, L], BF16, tag="qT")
                kT = qkpool.tile([P, NH, L], BF16, tag="kT")
                vN = vpool.tile([P, TT, HL], BF16, tag="vN")

                # =============== Phase A: projections =======================
                with (
                    tc.tile_pool(name="xT", bufs=1) as xTpool,
                    tc.tile_pool(name="wstr", bufs=3) as wpool,
                ):
                    # first q-weight tile before the bulk xT load
                    w_first = wpool.tile([P, NT, P], BF16, tag="wstr")
                    dma(w_first, wq[0])

                    xT = xTpool.tile([P, NT, L], BF16, tag="xT")
                    for ti in range(NT):
                        dma(xT[:, ti, :], xt[ti * P:(ti + 1) * P, :])

                    with tc.tile_pool(name="psA", bufs=4, space="PSUM") as psA:
                        for wt_d, bias_t, dest in ((wq, biasq, qT),
                                                   (wk, biask, kT)):
                            for dd in range(NH):
                                if wt_d is wq and dd == 0:
                                    w_sb = w_first
                                else:
                                    w_sb = wpool.tile([P, NT, P], BF16,
                                                      tag="wstr")
                                    dma(w_sb, wt_d[dd])
                                for c0 in range(0, L, CH):
                                    ps = psA.tile([P, CH], F32, tag="psA")
                                    for ki in range(NT):
                                        nc.tensor.matmul(
                                            ps, w_sb[:, ki, :],
                                            xT[:, ki, c0:c0 + CH],
                                            start=(ki == 0),
                                            stop=(ki == NT - 1))
                                    nc.vector.tensor_scalar_add(
                                        dest[:, dd, c0:c0 + CH], ps,
                                        bias_t[:, dd:dd + 1])

                    # v projection; xT stationary reused across dout halves
                    with (
                        tc.tile_pool(name="wv_sbp", bufs=1) as wvpool,
                        tc.tile_pool(name="psV", bufs=4, space="PSUM") as psV,
                    ):
                        wv_sb = wvpool.tile([P, NT, HL], BF16, tag="wv_sbp")
                        dma(wv_sb, wv[:, :, :])
                        for tt in range(TT):
                            ps0 = psV.tile([P, CH], F32, tag="psV")
                            ps1 = psV.tile([P, CH], F32, tag="psV")
                            for ki in range(NT):
                                nc.tensor.matmul(ps0,
                                                 xT[:, ki, tt * P:(tt + 1) * P],
                                                 wv_sb[:, ki, 0:CH],
                                                 start=(ki == 0),
                                                 stop=(ki == NT - 1))
                                nc.tensor.matmul(ps1,
                                                 xT[:, ki, tt * P:(tt + 1) * P],
                                                 wv_sb[:, ki, CH:HL],
                                                 start=(ki == 0),
                                                 stop=(ki == NT - 1))
                            nc.vector.tensor_copy(out=vN[:, tt, 0:CH], in_=ps0)
                            nc.vector.tensor_copy(out=vN[:, tt, CH:HL], in_=ps1)

                # =============== Phase B: attention =========================
                with (
                    tc.tile_pool(name="ao_sb", bufs=1) as aopool,
                    tc.tile_pool(name="ex", bufs=2) as expool,
                    tc.tile_pool(name="tr1", bufs=2) as tr1pool,
                    tc.tile_pool(name="tr2", bufs=2) as tr2pool,
                    tc.tile_pool(name="tr4", bufs=2) as tr4pool,
                    tc.tile_pool(name="rden", bufs=2) as rdenpool,
                    tc.tile_pool(name="psS", bufs=2, space="PSUM") as psS,
                    tc.tile_pool(name="psO", bufs=2, space="PSUM") as psO,
                    tc.tile_pool(name="psD", bufs=2, space="PSUM") as psD,
                ):
                    ao = aopool.tile([P, NH, L], BF16, tag="ao")
                    last_head_of = {g[-1]: i for i, g in enumerate(AG_GROUPS)}

                    def flush(pend):
                        """Denominator + normalize + v-bias for a finished
                        unit (delayed one unit)."""
                        t5, pso, h, c0 = pend
                        psd = psD.tile([P, CH], F32, tag="psD")
                        nc.tensor.matmul(psd[0:1, :], ones_c, t5,
                                         start=True, stop=True)
                        den = rdenpool.tile([1, CH], F32, tag="den")
                        nc.vector.tensor_copy(out=r(den), in_=psd[0:1, :])
                        psb = psD.tile([P, CH], F32, tag="psD")
                        nc.tensor.matmul(psb, r(ones_f[0:1, :]), r(den),
                                         start=True, stop=True)
                        rb = rdenpool.tile([P, CH], F32, tag="rb")
                        nc.vector.reciprocal(out=rb, in_=psb)
                        tmp = rdenpool.tile([P, CH], F32, tag="aotmp")
                        nc.vector.tensor_mul(tmp, pso, rb)
                        nc.vector.tensor_scalar_add(
                            ao[:, h, c0:c0 + CH], tmp, biasv[:, h:h + 1])
                        dma(ao_in[h * P:(h + 1) * P, c0:c0 + CH],
                            ao[:, h, c0:c0 + CH])
                        if c0 == L - CH and h in last_head_of:
                            g = last_head_of[h]
                            heads = AG_GROUPS[g]
                            nc.gpsimd.collective_compute(
                                "AllGather", mybir.AluOpType.bypass,
                                replica_groups=[list(range(NCORES))],
                                ins=[ao_in[heads[0] * P:
                                           (heads[-1] + 1) * P, :].opt()],
                                outs=[ags[g][:].opt()])

                    pending = None
                    for h in range(NH):
                        for c0 in range(0, L, CH):
                            ex = expool.tile([P, TT, CH], BF16, tag="ex")
                            pso = psO.tile([P, CH], F32, tag="psO")
                            # scores pair j interleaved with av of pair j-2
                            for j in range(TT // 2):
                                pss = psS.tile([P, 2, CH], F32, tag="psS")
                                for u in range(2):
                                    kt = 2 * j + u
                                    nc.tensor.matmul(
                                        pss[:, u, :],
                                        kT[:, h, kt * P:(kt + 1) * P],
                                        qT[:, h, c0:c0 + CH],
                                        start=True, stop=True)
                                nc.scalar.activation(ex[:, 2 * j:2 * j + 2, :],
                                                     pss, AF.Exp, scale=SCALE)
                                if j >= 2:
                                    for kt in (2 * j - 4, 2 * j - 3):
                                        nc.tensor.matmul(
                                            pso, vN[:, kt, h * P:(h + 1) * P],
                                            ex[:, kt, :],
                                            start=(kt == 0), stop=False)
                            for kt in range(TT - 4, TT):
                                nc.tensor.matmul(pso,
                                                 vN[:, kt, h * P:(h + 1) * P],
                                                 ex[:, kt, :],
                                                 start=False,
                                                 stop=(kt == TT - 1))
                            # denominator tree on the DVE
                            t1 = tr1pool.tile([P, 8, CH], BF16, tag="tr1")
                            nc.vector.tensor_add(t1, ex[:, 0:8, :],
                                                 ex[:, 8:16, :])
                            t2 = tr2pool.tile([P, 4, CH], BF16, tag="tr2")
                            nc.vector.tensor_add(t2, t1[:, 0:4, :],
                                                 t1[:, 4:8, :])
                            t4 = tr4pool.tile([P, 2, CH], BF16, tag="tr4")
                            nc.vector.tensor_add(t4, t2[:, 0:2, :],
                                                 t2[:, 2:4, :])
                            t5 = tr4pool.tile([P, CH], BF16, tag="tr5")
                            nc.vector.tensor_add(t5, t4[:, 0, :], t4[:, 1, :])

                            if pending is not None:
                                flush(pending)
                            pending = (t5, pso, h, c0)
                    flush(pending)

            # =============== Phase C: O projection ==========================
            # this core's 1024 tokens x all 2048 output dims, in two passes:
            # pass 1 contracts the core's OWN attn dims straight from its
            # local ao_in (no collective dependency -- runs while the last
            # AllGather lands); pass 2 adds the PEER half from the gathers,
            # selected with partition-id-derived dynamic DMA offsets.
            # wo rows are host-ordered [own 8 tiles, peer 8 tiles].
            peer_blk = blk0 + (pid + 1) % 2

            with (
                tc.tile_pool(name="wo_sbp", bufs=1) as wosbpool,
                tc.tile_pool(name="aoo", bufs=2) as aoopool,
                tc.tile_pool(name="ao2", bufs=2) as ao2pool,
                tc.tile_pool(name="partial", bufs=1) as partpool,
                tc.tile_pool(name="ost", bufs=3) as ostpool,
                tc.tile_pool(name="psC", bufs=4, space="PSUM") as psC,
            ):
                wo_sb = wosbpool.tile([P, NT, D], BF16, tag="wo_sbp")
                dma(wo_sb, wo[:, :, :])
                partial = partpool.tile([P, 2, NT, CH], F32, tag="partial")

                # ---- pass 1: own half (from local DRAM, AG-independent) ----
                for tci in range(2):
                    tc0 = tci * CH
                    aoo = aoopool.tile([P, NH, CH], BF16, tag="aoo")
                    for ht in range(NH):
                        dma(aoo[:, ht, :],
                            ao_in[ht * P:(ht + 1) * P,
                                  bass.ds(col0 + tc0, CH)])
                    for dd in range(NT):
                        ps = psC.tile([P, CH], F32, tag="psC")
                        for ht in range(NH):
                            nc.tensor.matmul(ps, wo_sb[:, ht, dd * P:(dd + 1) * P],
                                             aoo[:, ht, :],
                                             start=(ht == 0), stop=(ht == NH - 1))
                        nc.vector.tensor_copy(out=partial[:, tci, dd, :], in_=ps)

                # ---- pass 2: peer half from the gathers ----
                for tci in range(2):
                    tc0 = tci * CH
                    ao2 = ao2pool.tile([P, NH, CH], BF16, tag="ao2")
                    i = 0
                    for g, heads in enumerate(AG_GROUPS):
                        for hi in range(len(heads)):
                            src = ags[g][bass.ds(peer_blk, 1),
                                         hi * P:(hi + 1) * P,
                                         bass.ds(col0 + tc0, CH)]
                            dma(ao2[:, i:i + 1, :].transpose([0, 2, 1]),
                                src.transpose([1, 2, 0]))
                            i += 1
                    for dd in range(NT):
                        ps = psC.tile([P, CH], F32, tag="psC")
                        for ht in range(NH):
                            nc.tensor.matmul(ps,
                                             wo_sb[:, NH + ht, dd * P:(dd + 1) * P],
                                             ao2[:, ht, :],
                                             start=(ht == 0), stop=(ht == NH - 1))
                        o_sb = ostpool.tile([P, CH], F32, tag="ost")
                        nc.vector.scalar_tensor_tensor(
                            o_sb, ps, biaso[:, dd:dd + 1],
                            partial[:, tci, dd, :],
                            op0=mybir.AluOpType.add, op1=mybir.AluOpType.add)
                        dma(yt[dd * P:(dd + 1) * P, tc0:tc0 + CH], o_sb)

    nc.compile()
    return nc


def kernel(**inputs):
    inp = {k: np.asarray(v, dtype=np.float32) for k, v in inputs.items()}
    x = inp["x"]

    if "nc" not in _cache:
        _cache["nc"] = _build()
    nc = _cache["nc"]

    # fold LoRA into the dense weights: W' = W + SCALING * A @ B  (exact)
    wT = {}
    for p in "qkvo":
        Wp = inp[f"W{p}"] + SCALING * (inp[f"A{p}"] @ inp[f"B{p}"])
        wT[p] = np.ascontiguousarray(Wp.T)  # [din, dout] fp32

    def tile_qk(w):  # [D, HL] -> [NH, P, NT, P]
        return np.ascontiguousarray(
            w.reshape(NT, P, NH, P).transpose(2, 1, 0, 3)).astype(BF)

    def tile_v(w):  # [D, HL] -> [P, NT, HL]
        return np.ascontiguousarray(
            w.reshape(NT, P, HL).transpose(1, 0, 2)).astype(BF)

    def tile_o(hh):  # rows [own half, peer half] -> [P, NT, D]
        w = np.concatenate([wT["o"][hh * HL:(hh + 1) * HL],
                            wT["o"][(1 - hh) * HL:(2 - hh) * HL]])
        return np.ascontiguousarray(
            w.reshape(NT, P, D).transpose(1, 0, 2)).astype(BF)

    wo_t = [tile_o(0), tile_o(1)]

    in_maps = []
    for c in range(NCORES):
        b, hh = c // 2, c % 2
        S = slice(hh * HL, (hh + 1) * HL)
        m = {
            "xt": np.ascontiguousarray(x[b].T).astype(BF),
            "wq": tile_qk(wT["q"][:, S]),
            "wk": tile_qk(wT["k"][:, S]),
            "wv": tile_v(wT["v"][:, S]),
            "wo": wo_t[hh],
            "bq": np.ascontiguousarray(inp["bq"][S]),
            "bk": np.ascontiguousarray(inp["bk"][S]),
            "bv": np.ascontiguousarray(inp["bv"][S]),
            "bo": inp["bo"],
        }
        in_maps.append(m)

    trace = bool(int(os.environ.get("KERNEL_TRACE", "0")))
    res = run_bass_kernel_spmd(nc, in_maps, list(range(NCORES)), trace=trace)
    _cache["last_exec_time_ns"] = res.exec_time_ns
    _cache["last_result"] = res

    y = np.empty((B, L, D), dtype=np.float32)
    for c in range(NCORES):
        b, hh = c // 2, c % 2
        y[b, hh * LH:(hh + 1) * LH, :] = res.results[c]["yt"].T
    return y
